# revision 16
# baseline (speedup 1.0000x reference)
"""4-layer GCN encoder on 8 Trainium2 NeuronCores.

Strategy (graph/data parallel, dst-node sharding):
  - Nodes are permuted into 8*NB blocks of 128 (balanced by loopless
    in-degree) and sharded across 8 cores by destination.
  - Self-loop contributions are folded into the epilogue as a diagonal
    term (loopnorm * a_own), so edge tiles carry only the 800k real edges.
  - Every dense projection is computed on the local node shard only; the
    projected features are AllGathered (Shared-output collective) before
    each aggregation.
  - Aggregation: per-edge gather of source rows via SWDGE dma_gather (one
    merged call per (group, src-half)), then scatter-add on TensorE with
    S (one-hot * norm) as the stationary operand and the gathered messages
    M as the 256-wide moving operand: pb[dst, f] += S^T M.  One matmul per
    128-edge tile.
  - S depends only on the graph, so it is built once (layer 0) on the
    vector engine and spilled to DRAM; layers 1-3 stream it back via DMA.
  - h must be transposed (PE transpose via identity) for the next dense.
"""

import numpy as np
import ml_dtypes

import concourse.bacc as bacc
import concourse.mybir as mybir
import concourse.tile as tile
from concourse.bass_utils import run_bass_kernel_spmd

P = 128
BF16 = mybir.dt.bfloat16
F32 = mybir.dt.float32
I16 = mybir.dt.int16


class Cfg:
    def __init__(self, n_nodes=50000, n_edges=800000, in_ch=512, hid=256,
                 ncores=8, nb=49, G=7):
        self.n_nodes = n_nodes
        self.n_edges = n_edges
        self.in_ch = in_ch
        self.hid = hid
        self.ncores = ncores
        self.nb = nb                      # dst blocks of 128 per core
        self.G = G                        # blocks per gather group
        assert nb % G == 0
        self.NG = nb // G                 # groups per core
        self.shard = nb * P               # nodes per core (padded)
        self.npad = ncores * self.shard   # padded total nodes
        assert self.npad >= n_nodes
        self.half = self.npad // 2        # src-half boundary for int16 idx
        assert self.half % P == 0 and self.half < 32768
        self.fc_in = in_ch // P           # K chunks for layer 1
        self.fh = hid // P                # feature halves (2)
        assert self.fh == 2


CFG = Cfg()


# ----------------------------------------------------------------- host prep

def _preprocess(cfg, edge_index, edge_weight):
    """Numpy preprocessing: norms, balanced node permutation, per-core
    padded edge structures (self-loops excluded; handled as diagonal)."""
    N = cfg.n_nodes
    src = np.asarray(edge_index[0], dtype=np.int64)
    dst = np.asarray(edge_index[1], dtype=np.int64)
    ew = np.asarray(edge_weight, dtype=np.float32)
    # PyG gcn_norm with self-loops (fill 1): deg includes the +1 loop
    deg = np.bincount(dst, weights=ew.astype(np.float64), minlength=N)
    deg = deg.astype(np.float32) + 1.0
    dis = (1.0 / np.sqrt(deg)).astype(np.float32)
    norm = dis[src] * ew * dis[dst]
    loopnorm = dis * dis                       # self-loop: dis[v]*1*dis[v]

    # balanced block assignment: round-robin of loopless-degree-sorted nodes
    NBT = cfg.ncores * cfg.nb
    degc = np.bincount(dst, minlength=N)
    order = np.argsort(-degc, kind="stable")
    blk_of_rank = np.arange(N) % NBT
    pos_of_rank = np.arange(N) // NBT
    assert pos_of_rank.max() < P, "block capacity exceeded"
    gslot = np.empty(N, dtype=np.int64)
    gslot[order] = blk_of_rank * P + pos_of_rank

    ps = gslot[src]
    pd = gslot[dst]

    eb = pd // P                                  # global dst block per edge
    ehalf = (ps >= cfg.half).astype(np.int64)
    key = eb * 2 + ehalf
    cnt = np.bincount(key, minlength=NBT * 2)
    T = max(1, int(np.ceil(cnt.max() / P)))

    nslots = cfg.nb * 2 * T                       # tiles per core
    cap = nslots * P
    gidx16 = np.zeros((cfg.ncores, 16, cap // 16), dtype=np.int16)
    dstc = np.zeros((cfg.ncores, P, nslots), dtype=np.float32)
    normc = np.zeros((cfg.ncores, P, nslots), dtype=np.float32)

    core_e = eb // cfg.nb
    b_in_core = eb % cfg.nb
    g = b_in_core // cfg.G
    bg = b_in_core % cfg.G
    srt = np.lexsort((ps, ehalf, eb))             # (block, half, src) order
    key_s = key[srt]
    uniq, inv, counts = np.unique(key_s, return_inverse=True, return_counts=True)
    starts = np.zeros_like(counts)
    starts[1:] = np.cumsum(counts)[:-1]
    rank_in_bucket = np.arange(len(srt)) - starts[inv]

    es = srt
    t_idx = rank_in_bucket // P
    j_idx = rank_in_bucket % P
    assert t_idx.max() < T
    sg = g[es]
    sh = ehalf[es]
    sbg = bg[es]
    s_slot = ((sg * 2 + sh) * cfg.G + sbg) * T + t_idx
    q = s_slot * P + j_idx
    score = core_e[es]
    idxval = np.where(sh == 1, ps[es] - cfg.half, ps[es]).astype(np.int16)
    dlocal = (pd[es] % P).astype(np.float32)
    nval = norm[es]

    for c in range(cfg.ncores):
        m = score == c
        qc = q[m]
        gidx16[c, qc % 16, qc // 16] = idxval[m]
        dstc[c, qc % P, qc // P] = dlocal[m]
        normc[c, qc % P, qc // P] = nval[m]

    gidx = np.tile(gidx16, (1, 8, 1))             # replicate to 128 partitions

    # loopnorm per core: [P, nb] where [p, b] = loopnorm of permuted slot
    lpad = np.zeros(cfg.npad, np.float32)
    lpad[gslot] = loopnorm
    loopn = lpad.reshape(cfg.ncores, cfg.nb, P).transpose(0, 2, 1)
    return dict(T=T, nslots=nslots, gidx=gidx,
                dstc=dstc.astype(ml_dtypes.bfloat16),
                normc=normc.astype(ml_dtypes.bfloat16),
                loopn=np.ascontiguousarray(loopn),
                gslot=gslot)


def _pack_xts(cfg, x, gslot):
    """Per-core lhsT chunks for the layer-1 dense.
    xts[c] shape [nb*P, fc_in*P]: row bl*P + p, col fc*P + cnode =
    x_perm[node (c*nb + bl)*P + cnode, feature fc*P + p]."""
    xpad = np.zeros((cfg.npad, cfg.in_ch), dtype=np.float32)
    xpad[gslot] = x
    a = xpad.reshape(cfg.ncores, cfg.nb, P, cfg.fc_in, P)
    # [c, bl, cnode, fc, p] -> [c, bl, p, fc, cnode]
    a = a.transpose(0, 1, 4, 3, 2).reshape(cfg.ncores, cfg.nb * P, cfg.fc_in * P)
    return np.ascontiguousarray(a.astype(ml_dtypes.bfloat16))


def _pack_wcat(cfg, Ws):
    """[128, (fc_in + 3*fh)*hid] bf16 : W1 chunks then W2..W4 chunks."""
    cols = []
    for Wl in Ws:
        k = Wl.shape[0]
        for fc in range(k // P):
            cols.append(Wl[fc * P:(fc + 1) * P, :])
    return np.concatenate(cols, axis=1).astype(ml_dtypes.bfloat16)


def _iota_np():
    return np.tile(np.arange(P, dtype=np.float32)[None, :], (P, 1)).astype(
        ml_dtypes.bfloat16)


# ----------------------------------------------------------------- builder

def _build(cfg, T, n_layers=4, shared_ag=True, gather_ct=8):
    nslots = cfg.nb * 2 * T
    GT = cfg.G * T
    CT = gather_ct or GT                  # tiles per dma_gather call
    HID = cfg.hid
    nc = bacc.Bacc("TRN2", target_bir_lowering=False, debug=False,
                   num_devices=cfg.ncores, num_swdge_queues=4)
    qctr = [0]

    gidx_d = nc.dram_tensor("gidx", [P, nslots * 8], I16, kind="ExternalInput")
    dstc_d = nc.dram_tensor("dstc", [P, nslots], BF16, kind="ExternalInput")
    normc_d = nc.dram_tensor("normc", [P, nslots], BF16, kind="ExternalInput")
    iota_d = nc.dram_tensor("iota", [P, P], BF16, kind="ExternalInput")
    ident_d = nc.dram_tensor("ident", [P, P], BF16, kind="ExternalInput")
    wcat_cols = (cfg.fc_in + 3 * cfg.fh) * HID
    wcat_d = nc.dram_tensor("wcat", [P, wcat_cols], BF16, kind="ExternalInput")
    brep_d = nc.dram_tensor("brep", [P, 4 * HID], F32, kind="ExternalInput")
    arep_d = nc.dram_tensor("arep", [P, HID], F32, kind="ExternalInput")
    loopn_d = nc.dram_tensor("loopn", [P, cfg.nb], F32, kind="ExternalInput")
    xts_d = nc.dram_tensor("xts", [cfg.nb * P, cfg.fc_in * P], BF16,
                           kind="ExternalInput")
    out_d = nc.dram_tensor("out", [cfg.nb * P, HID], F32,
                           kind="ExternalOutput")

    w_off = {}
    off = 0
    for l in range(4):
        k = cfg.fc_in if l == 0 else cfg.fh
        for fc in range(k):
            w_off[(l, fc)] = off
            off += HID

    with tile.TileContext(nc) as tc:
        with (
            tc.tile_pool(name="res", bufs=1) as res,
            tc.tile_pool(name="mpool", bufs=2) as mpool,
            tc.tile_pool(name="spool", bufs=2) as spool,
            tc.tile_pool(name="xpool", bufs=2) as xpool,
            tc.tile_pool(name="apool", bufs=1) as apool,
            tc.tile_pool(name="hpool", bufs=4) as hpool,
            tc.tile_pool(name="htpool", bufs=1) as htpool,
            tc.tile_pool(name="opool", bufs=4) as opool,
            tc.tile_pool(name="ppool", bufs=cfg.G, space="PSUM") as ppool,
            tc.tile_pool(name="dram", bufs=2, space="DRAM") as dram,
            tc.tile_pool(name="dramsh", bufs=2, space="DRAM") as dramsh,
            tc.tile_pool(name="drams", bufs=1, space="DRAM") as drams,
        ):
            # ---- resident loads
            gidx = res.tile([P, nslots * 8], I16)
            nc.sync.dma_start(out=gidx[:], in_=gidx_d[:])
            dstc = res.tile([P, nslots], BF16)
            nc.sync.dma_start(out=dstc[:], in_=dstc_d[:])
            normc = res.tile([P, nslots], BF16)
            nc.sync.dma_start(out=normc[:], in_=normc_d[:])
            iota = res.tile([P, P], BF16)
            nc.sync.dma_start(out=iota[:], in_=iota_d[:])
            ident = res.tile([P, P], BF16)
            nc.sync.dma_start(out=ident[:], in_=ident_d[:])
            wcat = res.tile([P, wcat_cols], BF16)
            nc.sync.dma_start(out=wcat[:], in_=wcat_d[:])
            brep = res.tile([P, 4 * HID], F32)
            nc.sync.dma_start(out=brep[:], in_=brep_d[:])
            arep = res.tile([P, HID], F32)
            nc.sync.dma_start(out=arep[:], in_=arep_d[:])
            loopn = res.tile([P, cfg.nb], F32)
            nc.sync.dma_start(out=loopn[:], in_=loopn_d[:])

            sdump = drams.tile([P, 2 * cfg.NG * GT * P], BF16, tag="sdump",
                               name="sdump")

            hT = {}
            aown = {}

            def dense_block(l, nt, a_shard):
                """Dense projection for local block nt of layer l -> asb
                (kept in SBUF as aown[nt]) and DMA'd into a_shard."""
                pd_ = ppool.tile([P, HID], F32, tag="pb", name="pd")
                if l == 0:
                    xsl = xpool.tile([P, cfg.fc_in * P], BF16, tag="xsl",
                                     name="xsl")
                    nc.sync.dma_start(out=xsl[:],
                                      in_=xts_d[nt * P:(nt + 1) * P, :])
                    nk = cfg.fc_in
                    for fc in range(nk):
                        nc.tensor.matmul(
                            out=pd_[:],
                            lhsT=xsl[:, fc * P:(fc + 1) * P],
                            rhs=wcat[:, w_off[(0, fc)]:w_off[(0, fc)] + HID],
                            start=(fc == 0), stop=(fc == nk - 1))
                else:
                    for fc in range(cfg.fh):
                        nc.tensor.matmul(
                            out=pd_[:],
                            lhsT=hT[nt][:, fc * P:(fc + 1) * P],
                            rhs=wcat[:, w_off[(l, fc)]:w_off[(l, fc)] + HID],
                            start=(fc == 0), stop=(fc == cfg.fh - 1))
                asb = apool.tile([P, HID], BF16, tag=f"aown{nt}",
                                 name=f"aown{nt}")
                nc.scalar.copy(out=asb[:], in_=pd_[:])
                aown[nt] = asb
                nc.sync.dma_start(
                    out=a_shard[nt * P:(nt + 1) * P, :], in_=asb[:])

            def epilogue(l, nt, pbt):
                """pbt [dst, 256] f32 PSUM -> h (adds bias + self-loop term);
                l<3: also produce hT tiles for the next dense."""
                lt = hpool.tile([P, HID], F32, tag="lt", name="lt")
                nc.vector.tensor_scalar(
                    out=lt[:], in0=aown[nt][:],
                    scalar1=loopn[:, nt:nt + 1], scalar2=None,
                    op0=mybir.AluOpType.mult)
                hb = hpool.tile([P, HID], F32, tag="hb", name="hb")
                nc.vector.tensor_tensor(
                    out=hb[:], in0=pbt[:], in1=brep[:, l * HID:(l + 1) * HID],
                    op=mybir.AluOpType.add)
                if l < n_layers - 1:
                    hsb = hpool.tile([P, HID], BF16, tag="hsb", name="hsb")
                    nc.vector.tensor_tensor(
                        out=hsb[:], in0=hb[:], in1=lt[:],
                        op=mybir.AluOpType.add)
                    tp = ppool.tile([P, 2 * P], BF16, tag="pb", name="tp")
                    for fh in range(cfg.fh):
                        nc.tensor.transpose(
                            tp[:, fh * P:(fh + 1) * P],
                            hsb[:, fh * P:(fh + 1) * P], ident[:])
                    ht = htpool.tile([P, 2 * P], BF16, tag=f"hT{nt}",
                                     name=f"hT{nt}")
                    nc.scalar.copy(out=ht[:], in_=tp[:])
                    hT[nt] = ht
                else:
                    hb2 = hpool.tile([P, HID], F32, tag="hb2", name="hb2")
                    nc.vector.tensor_tensor(
                        out=hb2[:], in0=hb[:], in1=lt[:],
                        op=mybir.AluOpType.add)
                    neg = opool.tile([P, HID], F32, tag="neg", name="neg")
                    nc.vector.tensor_scalar(
                        out=neg[:], in0=hb2[:], scalar1=0.0, scalar2=None,
                        op0=mybir.AluOpType.min)
                    nega = opool.tile([P, HID], F32, tag="nega", name="nega")
                    nc.vector.tensor_tensor(
                        out=nega[:], in0=neg[:], in1=arep[:],
                        op=mybir.AluOpType.mult)
                    pos = opool.tile([P, HID], F32, tag="pos", name="pos")
                    nc.vector.tensor_scalar(
                        out=pos[:], in0=hb2[:], scalar1=0.0, scalar2=None,
                        op0=mybir.AluOpType.max)
                    osb = opool.tile([P, HID], F32, tag="osb", name="osb")
                    nc.vector.tensor_tensor(
                        out=osb[:], in0=pos[:], in1=nega[:],
                        op=mybir.AluOpType.add)
                    nc.sync.dma_start(
                        out=out_d[nt * P:(nt + 1) * P, :], in_=osb[:])

            def aggregate(l, a_full, a_shard_next):
                build_s = (l == 0)
                for g in range(cfg.NG):
                    pb = {}
                    Ms = {}
                    for h in range(2):
                        gh = g * 2 + h
                        M = mpool.tile([P, GT * HID], BF16, tag="M", name="M")
                        src_ap = (a_full[0:cfg.half, :] if h == 0
                                  else a_full[cfg.half:cfg.npad, :])
                        for k0 in range(0, GT, CT):
                            k1 = min(k0 + CT, GT)
                            nt_ = k1 - k0
                            nc.gpsimd.dma_gather(
                                out_ap=M[:, k0 * HID:k1 * HID].rearrange(
                                    "p (t e) -> p t e", e=HID),
                                in_ap=src_ap,
                                idxs_ap=gidx[:, (gh * GT + k0) * 8:
                                             (gh * GT + k1) * 8],
                                num_idxs=nt_ * P,
                                num_idxs_reg=nt_ * P,
                                elem_size=HID,
                                queue_num=qctr[0] % 4,
                            )
                            qctr[0] += 1
                        Ms[h] = M
                        S = spool.tile([P, GT * P], BF16, tag="S", name="S")
                        if build_s:
                            slot0 = gh * GT
                            s3 = S[:].rearrange("p (t e) -> p t e", e=P)
                            iob = iota[:].rearrange(
                                "p (o e) -> p o e", o=1).broadcast_to([P, GT, P])
                            nc.vector.tensor_tensor(
                                out=s3, in0=iob,
                                in1=dstc[:, slot0:slot0 + GT].to_broadcast(
                                    [P, GT, P]),
                                op=mybir.AluOpType.is_equal)
                            nc.vector.tensor_tensor(
                                out=s3, in0=s3,
                                in1=normc[:, slot0:slot0 + GT].to_broadcast(
                                    [P, GT, P]),
                                op=mybir.AluOpType.mult)
                            nc.sync.dma_start(
                                out=sdump[:, gh * GT * P:(gh + 1) * GT * P],
                                in_=S[:])
                        else:
                            nc.sync.dma_start(
                                out=S[:],
                                in_=sdump[:, gh * GT * P:(gh + 1) * GT * P])
                        for bg in range(cfg.G):
                            if h == 0:
                                pb[bg] = ppool.tile([P, HID], F32, tag="pb",
                                                    name="pb")
                            for t in range(T):
                                tl = bg * T + t
                                nc.tensor.matmul(
                                    out=pb[bg][:],
                                    lhsT=S[:, tl * P:(tl + 1) * P],
                                    rhs=M[:, tl * HID:(tl + 1) * HID],
                                    start=(h == 0 and t == 0),
                                    stop=(h == 1 and t == T - 1))
                    for bg in range(cfg.G):
                        nt = g * cfg.G + bg
                        epilogue(l, nt, pb[bg][:])
                        if l < n_layers - 1:
                            dense_block(l + 1, nt, a_shard_next)

            # ---- layer pipeline
            a_shard = dram.tile([cfg.shard, HID], BF16, tag="ashard",
                                name="ashard")
            for nt in range(cfg.nb):
                dense_block(0, nt, a_shard)
            for l in range(n_layers):
                a_full = dramsh.tile([cfg.npad, HID], BF16, tag="afull",
                                     name="afull",
                                     addr_space="Shared" if shared_ag else "Local")
                nc.gpsimd.collective_compute(
                    "AllGather",
                    mybir.AluOpType.bypass,
                    ins=[a_shard[:].opt()],
                    outs=[a_full[:].opt()],
                    replica_groups=[list(range(cfg.ncores))],
                )
                if l < n_layers - 1:
                    a_shard = dram.tile([cfg.shard, HID], BF16, tag="ashard",
                                        name="ashard")
                aggregate(l, a_full, a_shard)

    nc.compile()
    return nc


# ----------------------------------------------------------------- execution

def _make_in_maps(cfg, prep, x, Ws, bs, prelu_a):
    xts = _pack_xts(cfg, np.asarray(x, np.float32), prep["gslot"])
    wcat = _pack_wcat(cfg, Ws)
    brep = np.zeros((P, 4 * cfg.hid), np.float32)
    for l, b in enumerate(bs):
        brep[:, l * cfg.hid:(l + 1) * cfg.hid] = b[None, :]
    arep = np.tile(np.asarray(prelu_a, np.float32)[None, :], (P, 1))
    iota = _iota_np()
    ident = np.eye(P, dtype=ml_dtypes.bfloat16)
    maps = []
    for c in range(cfg.ncores):
        maps.append({
            "gidx": prep["gidx"][c],
            "dstc": prep["dstc"][c],
            "normc": prep["normc"][c],
            "iota": iota,
            "ident": ident,
            "wcat": wcat,
            "brep": brep,
            "arep": arep,
            "loopn": prep["loopn"][c],
            "xts": xts[c],
        })
    return maps


def _assemble_out(cfg, results, gslot):
    """results: per-core {'out': [nb*128, 256]} -> y [n_nodes, hid]."""
    yperm = np.concatenate([results[c]["out"] for c in range(cfg.ncores)],
                           axis=0)
    return np.ascontiguousarray(yperm[gslot]).astype(np.float32)


def run(cfg, x, edge_index, edge_weight, W1, b1, W2, b2, W3, b3, W4, b4,
        prelu_a, return_nc=False):
    prep = _preprocess(cfg, edge_index, edge_weight)
    nc = _build(cfg, prep["T"])
    in_maps = _make_in_maps(cfg, prep, x,
                            [np.asarray(W1, np.float32), np.asarray(W2, np.float32),
                             np.asarray(W3, np.float32), np.asarray(W4, np.float32)],
                            [np.asarray(b1, np.float32), np.asarray(b2, np.float32),
                             np.asarray(b3, np.float32), np.asarray(b4, np.float32)],
                            np.asarray(prelu_a, np.float32))
    res = run_bass_kernel_spmd(nc, in_maps, core_ids=list(range(cfg.ncores)))
    y = _assemble_out(cfg, res.results, prep["gslot"])
    if return_nc:
        return y, nc, in_maps
    return y


def kernel(x, edge_index, edge_weight, W1, b1, W2, b2, W3, b3, W4, b4, prelu_a):
    return run(CFG, x, edge_index, edge_weight,
               W1, b1, W2, b2, W3, b3, W4, b4, prelu_a)


# revision 27
# speedup vs baseline: 1.0734x; 1.0734x over previous
"""4-layer GCN encoder on 8 Trainium2 NeuronCores.

Strategy (graph/data parallel, dst-node sharding):
  - Nodes are permuted into 8*NB blocks of 128 (balanced by loopless
    in-degree) and sharded across 8 cores by destination.
  - Self-loop contributions are folded into the epilogue as a diagonal
    term (loopnorm * a_own), so edge tiles carry only the 800k real edges.
  - Every dense projection is computed on the local node shard only; the
    projected features are AllGathered (Shared-output collective) before
    each aggregation.
  - Aggregation: per-edge gather of source rows via SWDGE dma_gather (one
    merged call per (group, src-half)), then scatter-add on TensorE with
    S (one-hot * norm) as the stationary operand and the gathered messages
    M as the 256-wide moving operand: pb[dst, f] += S^T M.  One matmul per
    128-edge tile.
  - S depends only on the graph, so it is built once (layer 0) on the
    vector engine and spilled to DRAM; layers 1-3 stream it back via DMA.
  - h must be transposed (PE transpose via identity) for the next dense.
"""

import numpy as np
import ml_dtypes

import concourse.bacc as bacc
import concourse.mybir as mybir
import concourse.tile as tile
from concourse.bass_utils import run_bass_kernel_spmd

P = 128
BF16 = mybir.dt.bfloat16
F32 = mybir.dt.float32
I16 = mybir.dt.int16


class Cfg:
    def __init__(self, n_nodes=50000, n_edges=800000, in_ch=512, hid=256,
                 ncores=8, nb=49, G=7):
        self.n_nodes = n_nodes
        self.n_edges = n_edges
        self.in_ch = in_ch
        self.hid = hid
        self.ncores = ncores
        self.nb = nb                      # dst blocks of 128 per core
        self.G = G                        # blocks per gather group
        assert nb % G == 0
        self.NG = nb // G                 # groups per core
        self.shard = nb * P               # nodes per core (padded)
        self.npad = ncores * self.shard   # padded total nodes
        assert self.npad >= n_nodes
        self.half = self.npad // 2        # src-half boundary for int16 idx
        assert self.half % P == 0 and self.half < 32768
        self.fc_in = in_ch // P           # K chunks for layer 1
        self.fh = hid // P                # feature halves (2)
        assert self.fh == 2


CFG = Cfg()


# ----------------------------------------------------------------- host prep

def _preprocess(cfg, edge_index, edge_weight):
    """Numpy preprocessing: norms, balanced node permutation, per-core
    padded edge structures (self-loops excluded; handled as diagonal)."""
    N = cfg.n_nodes
    src = np.asarray(edge_index[0], dtype=np.int64)
    dst = np.asarray(edge_index[1], dtype=np.int64)
    ew = np.asarray(edge_weight, dtype=np.float32)
    # PyG gcn_norm with self-loops (fill 1): deg includes the +1 loop
    deg = np.bincount(dst, weights=ew.astype(np.float64), minlength=N)
    deg = deg.astype(np.float32) + 1.0
    dis = (1.0 / np.sqrt(deg)).astype(np.float32)
    norm = dis[src] * ew * dis[dst]
    loopnorm = dis * dis                       # self-loop: dis[v]*1*dis[v]

    # balanced block assignment: round-robin of loopless-degree-sorted nodes
    NBT = cfg.ncores * cfg.nb
    degc = np.bincount(dst, minlength=N)
    order = np.argsort(-degc, kind="stable")
    blk_of_rank = np.arange(N) % NBT
    pos_of_rank = np.arange(N) // NBT
    assert pos_of_rank.max() < P, "block capacity exceeded"
    gslot = np.empty(N, dtype=np.int64)
    gslot[order] = blk_of_rank * P + pos_of_rank

    ps = gslot[src]
    pd = gslot[dst]

    eb = pd // P                                  # global dst block per edge
    ehalf = (ps >= cfg.half).astype(np.int64)
    key = eb * 2 + ehalf
    cnt = np.bincount(key, minlength=NBT * 2)
    T = max(1, int(np.ceil(cnt.max() / P)))

    nslots = cfg.nb * 2 * T                       # tiles per core
    cap = nslots * P
    gidx16 = np.zeros((cfg.ncores, 16, cap // 16), dtype=np.int16)
    dstc = np.zeros((cfg.ncores, P, nslots), dtype=np.float32)
    normc = np.zeros((cfg.ncores, P, nslots), dtype=np.float32)

    core_e = eb // cfg.nb
    b_in_core = eb % cfg.nb
    g = b_in_core // cfg.G
    bg = b_in_core % cfg.G
    srt = np.lexsort((ps, ehalf, eb))             # (block, half, src) order
    key_s = key[srt]
    uniq, inv, counts = np.unique(key_s, return_inverse=True, return_counts=True)
    starts = np.zeros_like(counts)
    starts[1:] = np.cumsum(counts)[:-1]
    rank_in_bucket = np.arange(len(srt)) - starts[inv]

    es = srt
    t_idx = rank_in_bucket // P
    j_idx = rank_in_bucket % P
    assert t_idx.max() < T
    sg = g[es]
    sh = ehalf[es]
    sbg = bg[es]
    s_slot = ((sg * 2 + sh) * cfg.G + sbg) * T + t_idx
    q = s_slot * P + j_idx
    score = core_e[es]
    idxval = np.where(sh == 1, ps[es] - cfg.half, ps[es]).astype(np.int16)
    dlocal = (pd[es] % P).astype(np.float32)
    nval = norm[es]

    for c in range(cfg.ncores):
        m = score == c
        qc = q[m]
        gidx16[c, qc % 16, qc // 16] = idxval[m]
        dstc[c, qc % P, qc // P] = dlocal[m]
        normc[c, qc % P, qc // P] = nval[m]

    gidx = np.tile(gidx16, (1, 8, 1))             # replicate to 128 partitions

    # loopnorm per core: [P, nb] where [p, b] = loopnorm of permuted slot
    lpad = np.zeros(cfg.npad, np.float32)
    lpad[gslot] = loopnorm
    loopn = lpad.reshape(cfg.ncores, cfg.nb, P).transpose(0, 2, 1)
    return dict(T=T, nslots=nslots, gidx=gidx,
                dstc=dstc.astype(ml_dtypes.bfloat16),
                normc=normc.astype(ml_dtypes.bfloat16),
                loopn=np.ascontiguousarray(loopn),
                gslot=gslot)


def _pack_xts(cfg, x, gslot):
    """Per-core lhsT chunks for the layer-1 dense.
    xts[c] shape [nb*P, fc_in*P]: row bl*P + p, col fc*P + cnode =
    x_perm[node (c*nb + bl)*P + cnode, feature fc*P + p]."""
    xpad = np.zeros((cfg.npad, cfg.in_ch), dtype=np.float32)
    xpad[gslot] = x
    a = xpad.reshape(cfg.ncores, cfg.nb, P, cfg.fc_in, P)
    # [c, bl, cnode, fc, p] -> [c, bl, p, fc, cnode]
    a = a.transpose(0, 1, 4, 3, 2).reshape(cfg.ncores, cfg.nb * P, cfg.fc_in * P)
    return np.ascontiguousarray(a.astype(ml_dtypes.bfloat16))


def _pack_wcat(cfg, Ws):
    """[128, (fc_in + 3*fh)*hid] bf16 : W1 chunks then W2..W4 chunks."""
    cols = []
    for Wl in Ws:
        k = Wl.shape[0]
        for fc in range(k // P):
            cols.append(Wl[fc * P:(fc + 1) * P, :])
    return np.concatenate(cols, axis=1).astype(ml_dtypes.bfloat16)


def _iota_np():
    return np.tile(np.arange(P, dtype=np.float32)[None, :], (P, 1)).astype(
        ml_dtypes.bfloat16)


# ----------------------------------------------------------------- builder

def _build(cfg, T, n_layers=4, shared_ag=True, gather_ct=8):
    nslots = cfg.nb * 2 * T
    GT = cfg.G * T
    CT = gather_ct or GT                  # tiles per dma_gather call
    HID = cfg.hid
    nc = bacc.Bacc("TRN2", target_bir_lowering=False, debug=False,
                   num_devices=cfg.ncores, num_swdge_queues=4)
    qctr = [0]
    gsems = [nc.alloc_semaphore(f"gsem{q}") for q in range(8)]

    gidx_d = nc.dram_tensor("gidx", [P, nslots * 8], I16, kind="ExternalInput")
    dstc_d = nc.dram_tensor("dstc", [P, nslots], BF16, kind="ExternalInput")
    normc_d = nc.dram_tensor("normc", [P, nslots], BF16, kind="ExternalInput")
    iota_d = nc.dram_tensor("iota", [P, P], BF16, kind="ExternalInput")
    ident_d = nc.dram_tensor("ident", [P, P], BF16, kind="ExternalInput")
    wcat_cols = (cfg.fc_in + 3 * cfg.fh) * HID
    wcat_d = nc.dram_tensor("wcat", [P, wcat_cols], BF16, kind="ExternalInput")
    brep_d = nc.dram_tensor("brep", [P, 4 * HID], F32, kind="ExternalInput")
    arep_d = nc.dram_tensor("arep", [P, HID], F32, kind="ExternalInput")
    loopn_d = nc.dram_tensor("loopn", [P, cfg.nb], F32, kind="ExternalInput")
    xts_d = nc.dram_tensor("xts", [cfg.nb * P, cfg.fc_in * P], BF16,
                           kind="ExternalInput")
    out_d = nc.dram_tensor("out", [cfg.nb * P, HID], F32,
                           kind="ExternalOutput")

    w_off = {}
    off = 0
    for l in range(4):
        k = cfg.fc_in if l == 0 else cfg.fh
        for fc in range(k):
            w_off[(l, fc)] = off
            off += HID

    with tile.TileContext(nc) as tc:
        with (
            tc.tile_pool(name="res", bufs=1) as res,
            tc.tile_pool(name="mpool", bufs=2) as mpool,
            tc.tile_pool(name="spool", bufs=3) as spool,
            tc.tile_pool(name="xpool", bufs=1) as xpool,
            tc.tile_pool(name="apool", bufs=1) as apool,
            tc.tile_pool(name="hpool", bufs=3) as hpool,
            tc.tile_pool(name="htpool", bufs=1) as htpool,
            tc.tile_pool(name="opool", bufs=2) as opool,
            tc.tile_pool(name="ppool", bufs=cfg.G, space="PSUM") as ppool,
            tc.tile_pool(name="dram", bufs=2, space="DRAM") as dram,
            tc.tile_pool(name="dramsh", bufs=2, space="DRAM") as dramsh,
            tc.tile_pool(name="drams", bufs=1, space="DRAM") as drams,
        ):
            # ---- resident loads
            gidx = res.tile([P, nslots * 8], I16)
            nc.sync.dma_start(out=gidx[:], in_=gidx_d[:])
            dstc = res.tile([P, nslots], BF16)
            nc.sync.dma_start(out=dstc[:], in_=dstc_d[:])
            normc = res.tile([P, nslots], BF16)
            nc.sync.dma_start(out=normc[:], in_=normc_d[:])
            iota = res.tile([P, P], BF16)
            nc.sync.dma_start(out=iota[:], in_=iota_d[:])
            ident = res.tile([P, P], BF16)
            nc.sync.dma_start(out=ident[:], in_=ident_d[:])
            wcat = res.tile([P, wcat_cols], BF16)
            nc.sync.dma_start(out=wcat[:], in_=wcat_d[:])
            brep = res.tile([P, 4 * HID], F32)
            nc.sync.dma_start(out=brep[:], in_=brep_d[:])
            arep = res.tile([P, HID], F32)
            nc.sync.dma_start(out=arep[:], in_=arep_d[:])
            loopn = res.tile([P, cfg.nb], F32)
            nc.sync.dma_start(out=loopn[:], in_=loopn_d[:])

            sdump = drams.tile([P, 2 * cfg.NG * GT * P], BF16, tag="sdump",
                               name="sdump")

            hT = {}
            aown = {}

            def dense_block(l, nt, a_shard):
                """Dense projection for local block nt of layer l -> asb
                (kept in SBUF as aown[nt]) and DMA'd into a_shard."""
                pd_ = ppool.tile([P, HID], F32, tag="pb", name="pd")
                if l == 0:
                    xsl = xpool.tile([P, cfg.fc_in * P], BF16, tag="xsl",
                                     name="xsl")
                    nc.sync.dma_start(out=xsl[:],
                                      in_=xts_d[nt * P:(nt + 1) * P, :])
                    nk = cfg.fc_in
                    for fc in range(nk):
                        nc.tensor.matmul(
                            out=pd_[:],
                            lhsT=xsl[:, fc * P:(fc + 1) * P],
                            rhs=wcat[:, w_off[(0, fc)]:w_off[(0, fc)] + HID],
                            start=(fc == 0), stop=(fc == nk - 1))
                else:
                    for fc in range(cfg.fh):
                        nc.tensor.matmul(
                            out=pd_[:],
                            lhsT=hT[nt][:, fc * P:(fc + 1) * P],
                            rhs=wcat[:, w_off[(l, fc)]:w_off[(l, fc)] + HID],
                            start=(fc == 0), stop=(fc == cfg.fh - 1))
                asb = apool.tile([P, HID], BF16, tag=f"aown{nt}",
                                 name=f"aown{nt}")
                nc.scalar.copy(out=asb[:], in_=pd_[:])
                aown[nt] = asb
                nc.sync.dma_start(
                    out=a_shard[nt * P:(nt + 1) * P, :], in_=asb[:])

            def epilogue(l, nt, pbt):
                """pbt [dst, 256] f32 PSUM -> h (adds bias + self-loop term);
                l<3: also produce hT tiles for the next dense."""
                lt = hpool.tile([P, HID], F32, tag="lt", name="lt")
                nc.vector.tensor_scalar(
                    out=lt[:], in0=aown[nt][:],
                    scalar1=loopn[:, nt:nt + 1], scalar2=None,
                    op0=mybir.AluOpType.mult)
                hb = hpool.tile([P, HID], F32, tag="hb", name="hb")
                nc.vector.tensor_tensor(
                    out=hb[:], in0=pbt[:], in1=brep[:, l * HID:(l + 1) * HID],
                    op=mybir.AluOpType.add)
                if l < n_layers - 1:
                    hsb = hpool.tile([P, HID], BF16, tag="hsb", name="hsb")
                    nc.vector.tensor_tensor(
                        out=hsb[:], in0=hb[:], in1=lt[:],
                        op=mybir.AluOpType.add)
                    tp = ppool.tile([P, 2 * P], BF16, tag="pb", name="tp")
                    for fh in range(cfg.fh):
                        nc.tensor.transpose(
                            tp[:, fh * P:(fh + 1) * P],
                            hsb[:, fh * P:(fh + 1) * P], ident[:])
                    ht = htpool.tile([P, 2 * P], BF16, tag=f"hT{nt}",
                                     name=f"hT{nt}")
                    nc.scalar.copy(out=ht[:], in_=tp[:])
                    hT[nt] = ht
                else:
                    hb2 = hpool.tile([P, HID], F32, tag="hb2", name="hb2")
                    nc.vector.tensor_tensor(
                        out=hb2[:], in0=hb[:], in1=lt[:],
                        op=mybir.AluOpType.add)
                    neg = opool.tile([P, HID], F32, tag="neg", name="neg")
                    nc.vector.tensor_scalar(
                        out=neg[:], in0=hb2[:], scalar1=0.0, scalar2=None,
                        op0=mybir.AluOpType.min)
                    nega = opool.tile([P, HID], F32, tag="nega", name="nega")
                    nc.vector.tensor_tensor(
                        out=nega[:], in0=neg[:], in1=arep[:],
                        op=mybir.AluOpType.mult)
                    pos = opool.tile([P, HID], F32, tag="pos", name="pos")
                    nc.vector.tensor_scalar(
                        out=pos[:], in0=hb2[:], scalar1=0.0, scalar2=None,
                        op0=mybir.AluOpType.max)
                    osb = opool.tile([P, HID], F32, tag="osb", name="osb")
                    nc.vector.tensor_tensor(
                        out=osb[:], in0=pos[:], in1=nega[:],
                        op=mybir.AluOpType.add)
                    nc.sync.dma_start(
                        out=out_d[nt * P:(nt + 1) * P, :], in_=osb[:])

            def aggregate(l, a_full, a_shard_next):
                build_s = (l == 0)
                for g in range(cfg.NG):
                    pb = {}
                    Ms = {}
                    for h in range(2):
                        gh = g * 2 + h
                        M = mpool.tile([P, GT * HID], BF16, tag="M", name="M")
                        src_ap = (a_full[0:cfg.half, :] if h == 0
                                  else a_full[cfg.half:cfg.npad, :])
                        for k0 in range(0, GT, CT):
                            k1 = min(k0 + CT, GT)
                            nt_ = k1 - k0
                            nc.gpsimd.dma_gather(
                                out_ap=M[:, k0 * HID:k1 * HID].rearrange(
                                    "p (t e) -> p t e", e=HID),
                                in_ap=src_ap,
                                idxs_ap=gidx[:, (gh * GT + k0) * 8:
                                             (gh * GT + k1) * 8],
                                num_idxs=nt_ * P,
                                num_idxs_reg=nt_ * P,
                                elem_size=HID,
                                queue_num=qctr[0] % 4,
                            )
                            qctr[0] += 1
                        Ms[h] = M
                        S = spool.tile([P, GT * P], BF16, tag="S", name="S")
                        if build_s:
                            slot0 = gh * GT
                            s3 = S[:].rearrange("p (t e) -> p t e", e=P)
                            iob = iota[:].rearrange(
                                "p (o e) -> p o e", o=1).broadcast_to([P, GT, P])
                            nc.vector.tensor_tensor(
                                out=s3, in0=iob,
                                in1=dstc[:, slot0:slot0 + GT].to_broadcast(
                                    [P, GT, P]),
                                op=mybir.AluOpType.is_equal)
                            nc.vector.tensor_tensor(
                                out=s3, in0=s3,
                                in1=normc[:, slot0:slot0 + GT].to_broadcast(
                                    [P, GT, P]),
                                op=mybir.AluOpType.mult)
                            nc.sync.dma_start(
                                out=sdump[:, gh * GT * P:(gh + 1) * GT * P],
                                in_=S[:])
                        else:
                            nc.sync.dma_start(
                                out=S[:],
                                in_=sdump[:, gh * GT * P:(gh + 1) * GT * P])
                        for bg in range(cfg.G):
                            if h == 0:
                                pb[bg] = ppool.tile([P, HID], F32, tag="pb",
                                                    name="pb")
                            for t in range(T):
                                tl = bg * T + t
                                nc.tensor.matmul(
                                    out=pb[bg][:],
                                    lhsT=S[:, tl * P:(tl + 1) * P],
                                    rhs=M[:, tl * HID:(tl + 1) * HID],
                                    start=(h == 0 and t == 0),
                                    stop=(h == 1 and t == T - 1))
                    for bg in range(cfg.G):
                        nt = g * cfg.G + bg
                        epilogue(l, nt, pb[bg][:])
                        if l < n_layers - 1:
                            dense_block(l + 1, nt, a_shard_next)

            # ---- layer pipeline
            a_shard = dram.tile([cfg.shard, HID], BF16, tag="ashard",
                                name="ashard")
            for nt in range(cfg.nb):
                dense_block(0, nt, a_shard)
            for l in range(n_layers):
                a_full = dramsh.tile([cfg.npad, HID], BF16, tag="afull",
                                     name="afull",
                                     addr_space="Shared" if shared_ag else "Local")
                nc.gpsimd.collective_compute(
                    "AllGather",
                    mybir.AluOpType.bypass,
                    ins=[a_shard[:].opt()],
                    outs=[a_full[:].opt()],
                    replica_groups=[list(range(cfg.ncores))],
                )
                if l < n_layers - 1:
                    a_shard = dram.tile([cfg.shard, HID], BF16, tag="ashard",
                                        name="ashard")
                aggregate(l, a_full, a_shard)

    nc.compile()
    return nc


# ----------------------------------------------------------------- execution

def _make_in_maps(cfg, prep, x, Ws, bs, prelu_a):
    xts = _pack_xts(cfg, np.asarray(x, np.float32), prep["gslot"])
    wcat = _pack_wcat(cfg, Ws)
    brep = np.zeros((P, 4 * cfg.hid), np.float32)
    for l, b in enumerate(bs):
        brep[:, l * cfg.hid:(l + 1) * cfg.hid] = b[None, :]
    arep = np.tile(np.asarray(prelu_a, np.float32)[None, :], (P, 1))
    iota = _iota_np()
    ident = np.eye(P, dtype=ml_dtypes.bfloat16)
    maps = []
    for c in range(cfg.ncores):
        maps.append({
            "gidx": prep["gidx"][c],
            "dstc": prep["dstc"][c],
            "normc": prep["normc"][c],
            "iota": iota,
            "ident": ident,
            "wcat": wcat,
            "brep": brep,
            "arep": arep,
            "loopn": prep["loopn"][c],
            "xts": xts[c],
        })
    return maps


def _assemble_out(cfg, results, gslot):
    """results: per-core {'out': [nb*128, 256]} -> y [n_nodes, hid]."""
    yperm = np.concatenate([results[c]["out"] for c in range(cfg.ncores)],
                           axis=0)
    return np.ascontiguousarray(yperm[gslot]).astype(np.float32)


def run(cfg, x, edge_index, edge_weight, W1, b1, W2, b2, W3, b3, W4, b4,
        prelu_a, return_nc=False):
    prep = _preprocess(cfg, edge_index, edge_weight)
    nc = _build(cfg, prep["T"])
    in_maps = _make_in_maps(cfg, prep, x,
                            [np.asarray(W1, np.float32), np.asarray(W2, np.float32),
                             np.asarray(W3, np.float32), np.asarray(W4, np.float32)],
                            [np.asarray(b1, np.float32), np.asarray(b2, np.float32),
                             np.asarray(b3, np.float32), np.asarray(b4, np.float32)],
                            np.asarray(prelu_a, np.float32))
    res = run_bass_kernel_spmd(nc, in_maps, core_ids=list(range(cfg.ncores)))
    y = _assemble_out(cfg, res.results, prep["gslot"])
    if return_nc:
        return y, nc, in_maps
    return y


def kernel(x, edge_index, edge_weight, W1, b1, W2, b2, W3, b3, W4, b4, prelu_a):
    return run(CFG, x, edge_index, edge_weight,
               W1, b1, W2, b2, W3, b3, W4, b4, prelu_a)
